# revision 14
# baseline (speedup 1.0000x reference)
"""ESM2 contact predictor head on 8 Trainium2 NeuronCores.

Computes out[b, i, j] = sigmoid(x[b,i] @ W @ x[b,j] + bias) for
x: (8, 2050, 320) f32, W: (320, 320) f32, bias: (1,) f32.

Sharding: data-parallel over batch — core c handles batch element c.

Decomposition (host + device):
  host:  SVD-rotate the bilinear form: W = U S V^T, A = x @ (U sqrt(S)),
         B = x @ (V sqrt(S)), so logits = A @ B^T. Host also computes the
         2 tail rows (i=2048:2050) and 2 tail cols (j=2048:2050) exactly,
         so the device handles a clean 2048x2048 grid.
  chip:  per core, out[0:2048, 0:2048] = sigmoid(A @ B^T + b) in bf16
         (K=320 as 2 x 128-mode slabs + 1 x 64-mode slab, all bf16 at
         1 col/cycle; fp8 DoubleRow was tried and pins the PE clock at
         the 1.2 GHz mid pstate — net loss).
         All input lands via 3 large DMAs into one flat SBUF buffer
         (per-DMA overhead ~1us makes small chunks wire-inefficient).
         13 warmup matmuls bridge the input-DMA window so the PE stream
         never gaps (a gap resets the 2.4 GHz pstate ramp). Strips 0-3
         are scheduled in input-arrival order (j-groups 0-1 first, the
         64-mode slab and j-groups 2-3 after the second DMA). PSUM is
         managed as 8 single-bank [128,512] regions; matmuls touching
         the same region are >=4 apart to hide the accumulate RMW
         turnaround. Fused sigmoid+bias on ScalarE writes bf16 strips;
         one 0.5 MB DMA out per 128-row strip (split for the last strip
         so the final drain is short).
"""

import numpy as np

import concourse.mybir as mybir
import concourse.tile as tile
from concourse import bacc
from concourse.bass_utils import run_bass_kernel_spmd

N_CORES = 8
B, L, D = 8, 2050, 320
LC = 2048          # device output grid (tails handled on host)
F32 = mybir.dt.float32
BF16 = mybir.dt.bfloat16
SIG = mybir.ActivationFunctionType.Sigmoid

# flat input layout, [128, FLAT] bf16 elements per partition:
#   [0:2048)      u2A: stationary A^T modes 0:256 as (k, i), i in 0:1024
#   [2048:3072)   x2 g0: moving B^T modes 0:256 as (k, j), j in 0:512
#   [3072:4096)   x2 g1
#   [4096:5120)   x2 g2
#   [5120:6144)   x2 g3
#   [6144:8192)   u2B: stationary, i in 1024:2048
# plus a [64, 4096] tensor for the 64-mode slab (lhsT/rhs must share a
# base partition): [0:2048) = A^T modes 256:320, [2048:4096) = B^T.
FLAT = 8192
OFF_U2 = (0, 6144)
OFF_G = (2048, 3072, 4096, 5120)

_cache = {}


def _build(bias_val: float):
    nc = bacc.Bacc("TRN2", target_bir_lowering=False, debug=False,
                   num_devices=N_CORES)
    in_d = nc.dram_tensor("inflat", [128, FLAT], BF16, kind="ExternalInput")
    ux3_d = nc.dram_tensor("ux3", [64, 4096], BF16, kind="ExternalInput")
    out_d = nc.dram_tensor("out", [LC, LC], BF16, kind="ExternalOutput")

    with tile.TileContext(nc) as tc:
        with (
            tc.tile_pool(name="persist", bufs=1) as pp,
            tc.tile_pool(name="outp", bufs=4) as outp,
            tc.tile_pool(name="psum", bufs=8, space="PSUM") as psp,
        ):
            bias_t = pp.tile([128, 1], F32)
            nc.vector.memset(bias_t[:], bias_val)

            inbuf = pp.tile([128, FLAT], BF16)
            ux3buf = pp.tile([64, 4096], BF16)

            def u2ap(k, i0):
                ih, off = divmod(i0, 1024)
                base = OFF_U2[ih] + k * 1024 + off
                return inbuf[:, base:base + 128]

            def x2ap(g, k):
                base = OFF_G[g] + k * 512
                return inbuf[:, base:base + 512]

            def u3ap(i0):
                return ux3buf[:, i0:i0 + 128]

            def x3ap(g):
                return ux3buf[:, 2048 + g * 512:2048 + (g + 1) * 512]

            # input DMAs split across the two HWDGE queues (SP + Activation);
            # each queue wires ~160 GB/s, so pairing them halves the lead-in
            nc.sync.dma_start(inbuf[:, 0:2048], in_d.ap()[:, 0:2048])
            nc.scalar.dma_start(inbuf[:, 2048:4096], in_d.ap()[:, 2048:4096])
            nc.sync.dma_start(ux3buf[:], ux3_d.ap())
            nc.scalar.dma_start(inbuf[:, 4096:6144], in_d.ap()[:, 4096:6144])
            nc.sync.dma_start(inbuf[:, 6144:FLAT], in_d.ap()[:, 6144:FLAT])

            # PE warmup: release the HAM clock-gate and hold the pstate ramp
            # while DMA 1 (u2A + j-groups 0-1) lands.
            warm_sb = pp.tile([128, 512], BF16)
            nc.vector.memset(warm_sb.bitcast(F32)[:], 1.0)
            psw = [psp.tile([128, 512], F32, tag="reg", bufs=8,
                            name=f"psw{i}") for i in range(2)]
            for wi in range(9):
                nc.tensor.matmul(psw[wi % 2][:, :], lhsT=warm_sb[:, :128],
                                 rhs=warm_sb[:], start=True, stop=True)
            # preload the sigmoid activation table while DMAs run
            act_warm = pp.tile([128, 1], F32)
            nc.scalar.activation(act_warm[:], bias_t[:], SIG)

            strips = {}
            regs = {}

            def reg(s, r):
                if (s, r) not in regs:
                    regs[(s, r)] = psp.tile([128, 512], F32, tag="reg",
                                            bufs=8, name=f"r{s}_{r}")
                return regs[(s, r)]

            def mm2(s, r, k):
                nc.tensor.matmul(reg(s, r)[:, :], lhsT=u2ap(k, s * 128),
                                 rhs=x2ap(r, k), start=(k == 0), stop=False)

            def mm3(s, r):
                nc.tensor.matmul(reg(s, r)[:, :], lhsT=u3ap(s * 128),
                                 rhs=x3ap(r), start=False, stop=True)

            def act(s, r):
                nc.scalar.activation(strips[s][:, r * 512:(r + 1) * 512],
                                     reg(s, r)[:, :], SIG, bias=bias_t[:, :])
                del regs[(s, r)]

            def dma_out(s, r0, r1, eng=None):
                i0 = s * 128
                eng = eng or (nc.sync if s % 2 == 0 else nc.scalar)
                eng.dma_start(
                    out_d.ap()[i0:i0 + 128, r0 * 512:r1 * 512],
                    strips[s][:, r0 * 512:r1 * 512])

            for s in range(4):
                strips[s] = outp.tile([128, LC], BF16, tag="strip", bufs=4,
                                      name=f"strip{s}")
            # lead-in: strips 0-3 in input-arrival order.
            # B1: j-groups 0-1 (needs DMA 1 only), distance-8
            for k in range(2):
                for s in range(4):
                    for r in range(2):
                        mm2(s, r, k)
            # B2: 64-mode slab for those regions (needs DMA 2), then sigmoid
            for s in range(4):
                for r in range(2):
                    mm3(s, r)
                    act(s, r)
            # B3: j-groups 2-3 (needs DMA 2), distance-8, then close strips
            for k in range(2):
                for s in range(4):
                    for r in range(2, 4):
                        mm2(s, r, k)
            for s in range(4):
                for r in range(2, 4):
                    mm3(s, r)
                    act(s, r)
                dma_out(s, 0, 4)

            # steady state: strips 4-15, distance-4 within a strip
            for s in range(4, 16):
                strips[s] = outp.tile([128, LC], BF16, tag="strip", bufs=4,
                                      name=f"strip{s}")
                for k in range(2):
                    for r in range(4):
                        mm2(s, r, k)
                for r in range(4):
                    mm3(s, r)
                    act(s, r)
                    if s == 15 and r == 1:
                        # last strip: halves drain on both queues in parallel
                        dma_out(s, 0, 2, eng=nc.scalar)
                if s == 15:
                    dma_out(s, 2, 4, eng=nc.sync)
                else:
                    dma_out(s, 0, 4)

    nc.compile()
    return nc


last_results = None


def _sigmoid(z):
    return 1.0 / (1.0 + np.exp(-z))


def _host_pack(x, W):
    U, s, Vt = np.linalg.svd(W.astype(np.float64))
    A = np.ascontiguousarray(x @ (U * np.sqrt(s)).astype(np.float32))
    Bm = np.ascontiguousarray(x @ (Vt.T * np.sqrt(s)).astype(np.float32))

    flat = np.empty((B, 128, FLAT), np.float32)
    A2T = A[:, :LC, :256].transpose(0, 2, 1)       # (B, 256, 2048)
    B2T = Bm[:, :LC, :256].transpose(0, 2, 1)
    for k in range(2):
        sl = A2T[:, k * 128:(k + 1) * 128]
        flat[:, :, k * 1024:(k + 1) * 1024] = sl[:, :, 0:1024]
        flat[:, :, 6144 + k * 1024:6144 + (k + 1) * 1024] = sl[:, :, 1024:2048]
        for g in range(4):
            flat[:, :, OFF_G[g] + k * 512:OFF_G[g] + (k + 1) * 512] = \
                B2T[:, k * 128:(k + 1) * 128, g * 512:(g + 1) * 512]
    ux3 = np.empty((B, 64, 4096), np.float32)
    ux3[:, :, 0:LC] = A[:, :LC, 256:].transpose(0, 2, 1)
    ux3[:, :, LC:] = Bm[:, :LC, 256:].transpose(0, 2, 1)
    bf = mybir.dt.np(BF16)
    return A, Bm, flat.astype(bf), ux3.astype(bf)


def kernel(x, W, b, _trace=False):
    global last_results
    x = np.ascontiguousarray(np.asarray(x, dtype=np.float32))
    W = np.asarray(W, dtype=np.float32)
    b = np.asarray(b, dtype=np.float32)
    bias_val = float(b[0])

    if bias_val not in _cache:
        _cache.clear()
        _cache[bias_val] = _build(bias_val)
    nc = _cache[bias_val]

    A, Bm, flat, ux3 = _host_pack(x, W)
    in_maps = [{"inflat": flat[c], "ux3": ux3[c]} for c in range(N_CORES)]
    res = run_bass_kernel_spmd(nc, in_maps, core_ids=list(range(N_CORES)),
                               trace=_trace)
    last_results = res

    out = np.empty((B, L, L), dtype=np.float32)
    for c in range(N_CORES):
        out[c, :LC, :LC] = res.results[c]["out"].astype(np.float32)
    # host-computed tails: rows 2048:2050 (all j) and cols 2048:2050
    row_logits = np.einsum('bid,bjd->bij', A[:, LC:], Bm, optimize=True)
    col_logits = np.einsum('bid,bjd->bij', A[:, :LC], Bm[:, LC:],
                           optimize=True)
    out[:, LC:, :] = _sigmoid(row_logits + bias_val)
    out[:, :LC, LC:] = _sigmoid(col_logits + bias_val)
    return out


# revision 18
# speedup vs baseline: 1.0416x; 1.0416x over previous
"""ESM2 contact predictor head on 8 Trainium2 NeuronCores.

Computes out[b, i, j] = sigmoid(x[b,i] @ W @ x[b,j] + bias) for
x: (8, 2050, 320) f32, W: (320, 320) f32, bias: (1,) f32.

Sharding: data-parallel over batch — core c handles batch element c.

Decomposition (host + device):
  host:  SVD-rotate the bilinear form: W = U S V^T, A = x @ (U sqrt(S)),
         B = x @ (V sqrt(S)), so logits = A @ B^T. Host also computes the
         2 tail rows (i=2048:2050) and 2 tail cols (j=2048:2050) exactly,
         so the device handles a clean 2048x2048 grid.
  chip:  per core, out[0:2048, 0:2048] = sigmoid(A @ B^T + b) in bf16
         (K=320 as 2 x 128-mode slabs + 1 x 64-mode slab, all bf16 at
         1 col/cycle; fp8 DoubleRow was tried and pins the PE clock at
         the 1.2 GHz mid pstate — net loss).
         All input lands via 3 large DMAs into one flat SBUF buffer
         (per-DMA overhead ~1us makes small chunks wire-inefficient).
         13 warmup matmuls bridge the input-DMA window so the PE stream
         never gaps (a gap resets the 2.4 GHz pstate ramp). Strips 0-3
         are scheduled in input-arrival order (j-groups 0-1 first, the
         64-mode slab and j-groups 2-3 after the second DMA). PSUM is
         managed as 8 single-bank [128,512] regions; matmuls touching
         the same region are >=4 apart to hide the accumulate RMW
         turnaround. Fused sigmoid+bias on ScalarE writes bf16 strips;
         one 0.5 MB DMA out per 128-row strip (split for the last strip
         so the final drain is short).
"""

import numpy as np

import concourse.mybir as mybir
import concourse.tile as tile
from concourse import bacc
from concourse.bass_utils import run_bass_kernel_spmd

N_CORES = 8
B, L, D = 8, 2050, 320
LC = 2048          # device output grid (tails handled on host)
F32 = mybir.dt.float32
BF16 = mybir.dt.bfloat16
SIG = mybir.ActivationFunctionType.Sigmoid

# flat input layout, [128, FLAT] bf16 elements per partition:
#   [0:2048)      u2A: stationary A^T modes 0:256 as (k, i), i in 0:1024
#   [2048:3072)   x2 g0: moving B^T modes 0:256 as (k, j), j in 0:512
#   [3072:4096)   x2 g1
#   [4096:5120)   x2 g2
#   [5120:6144)   x2 g3
#   [6144:8192)   u2B: stationary, i in 1024:2048
# plus a [64, 4096] tensor for the 64-mode slab (lhsT/rhs must share a
# base partition): [0:2048) = A^T modes 256:320, [2048:4096) = B^T.
FLAT = 8192
OFF_U2 = (0, 6144)
OFF_G = (2048, 3072, 4096, 5120)

_cache = {}


def _build(bias_val: float):
    nc = bacc.Bacc("TRN2", target_bir_lowering=False, debug=False,
                   num_devices=N_CORES)
    in_d = nc.dram_tensor("inflat", [128, FLAT], BF16, kind="ExternalInput")
    ux3_d = nc.dram_tensor("ux3", [64, 4096], BF16, kind="ExternalInput")
    out_d = nc.dram_tensor("out", [LC, LC], BF16, kind="ExternalOutput")

    with tile.TileContext(nc) as tc:
        with (
            tc.tile_pool(name="persist", bufs=1) as pp,
            tc.tile_pool(name="outp", bufs=4) as outp,
            tc.tile_pool(name="psum", bufs=8, space="PSUM") as psp,
        ):
            bias_t = pp.tile([128, 1], F32)
            nc.vector.memset(bias_t[:], bias_val)

            inbuf = pp.tile([128, FLAT], BF16)
            ux3buf = pp.tile([64, 4096], BF16)

            def u2ap(k, i0):
                ih, off = divmod(i0, 1024)
                base = OFF_U2[ih] + k * 1024 + off
                return inbuf[:, base:base + 128]

            def x2ap(g, k):
                base = OFF_G[g] + k * 512
                return inbuf[:, base:base + 512]

            def u3ap(i0):
                return ux3buf[:, i0:i0 + 128]

            def x3ap(g):
                return ux3buf[:, 2048 + g * 512:2048 + (g + 1) * 512]

            # 4 large input DMAs on the SP queue (per-DMA overhead ~1us;
            # a second queue on the Activation engine evicts the sigmoid
            # table and delays ACTs — measured net loss)
            nc.sync.dma_start(inbuf[:, 0:4096], in_d.ap()[:, 0:4096])
            nc.sync.dma_start(ux3buf[:], ux3_d.ap())
            nc.sync.dma_start(inbuf[:, 4096:6144], in_d.ap()[:, 4096:6144])
            nc.sync.dma_start(inbuf[:, 6144:FLAT], in_d.ap()[:, 6144:FLAT])

            # PE warmup: release the HAM clock-gate and hold the pstate ramp
            # while DMA 1 (u2A + j-groups 0-1) lands.
            warm_sb = pp.tile([128, 512], BF16)
            nc.vector.memset(warm_sb.bitcast(F32)[:], 1.0)
            psw = [psp.tile([128, 512], F32, tag="reg", bufs=8,
                            name=f"psw{i}") for i in range(2)]
            for wi in range(15):
                nc.tensor.matmul(psw[wi % 2][:, :], lhsT=warm_sb[:, :128],
                                 rhs=warm_sb[:], start=True, stop=True)
            # preload the sigmoid activation table while DMAs run
            act_warm = pp.tile([128, 1], F32)
            nc.scalar.activation(act_warm[:], bias_t[:], SIG)

            strips = {}
            regs = {}

            def reg(s, r):
                if (s, r) not in regs:
                    regs[(s, r)] = psp.tile([128, 512], F32, tag="reg",
                                            bufs=8, name=f"r{s}_{r}")
                return regs[(s, r)]

            def mm2(s, r, k):
                nc.tensor.matmul(reg(s, r)[:, :], lhsT=u2ap(k, s * 128),
                                 rhs=x2ap(r, k), start=(k == 0), stop=False)

            def mm3(s, r):
                nc.tensor.matmul(reg(s, r)[:, :], lhsT=u3ap(s * 128),
                                 rhs=x3ap(r), start=False, stop=True)

            def act(s, r):
                nc.scalar.activation(strips[s][:, r * 512:(r + 1) * 512],
                                     reg(s, r)[:, :], SIG, bias=bias_t[:, :])
                del regs[(s, r)]

            def dma_out(s, r0, r1, eng=None):
                i0 = s * 128
                (eng or nc.sync).dma_start(
                    out_d.ap()[i0:i0 + 128, r0 * 512:r1 * 512],
                    strips[s][:, r0 * 512:r1 * 512])

            for s in range(4):
                strips[s] = outp.tile([128, LC], BF16, tag="strip", bufs=4,
                                      name=f"strip{s}")
            # lead-in: strips 0-3 in input-arrival order.
            # B1: j-groups 0-1 (needs DMA 1 only), distance-8
            for k in range(2):
                for s in range(4):
                    for r in range(2):
                        mm2(s, r, k)
            # B2: 64-mode slab for those regions (needs DMA 2), then sigmoid
            for s in range(4):
                for r in range(2):
                    mm3(s, r)
                    act(s, r)
            # B3: j-groups 2-3 (needs DMA 2), distance-8, then close strips
            for k in range(2):
                for s in range(4):
                    for r in range(2, 4):
                        mm2(s, r, k)
            for s in range(4):
                for r in range(2, 4):
                    mm3(s, r)
                    act(s, r)
                dma_out(s, 0, 4)

            # steady state: strips 4-15, distance-4 within a strip
            for s in range(4, 16):
                strips[s] = outp.tile([128, LC], BF16, tag="strip", bufs=4,
                                      name=f"strip{s}")
                for k in range(2):
                    for r in range(4):
                        mm2(s, r, k)
                for r in range(4):
                    mm3(s, r)
                    act(s, r)
                    if s == 15 and r == 1:
                        dma_out(s, 0, 2)   # split the last strip's DMA
                if s == 15:
                    dma_out(s, 2, 4, eng=nc.sync)
                else:
                    dma_out(s, 0, 4)

    nc.compile()
    return nc


last_results = None


def _sigmoid(z):
    return 1.0 / (1.0 + np.exp(-z))


def _host_pack(x, W):
    U, s, Vt = np.linalg.svd(W.astype(np.float64))
    A = np.ascontiguousarray(x @ (U * np.sqrt(s)).astype(np.float32))
    Bm = np.ascontiguousarray(x @ (Vt.T * np.sqrt(s)).astype(np.float32))

    flat = np.empty((B, 128, FLAT), np.float32)
    A2T = A[:, :LC, :256].transpose(0, 2, 1)       # (B, 256, 2048)
    B2T = Bm[:, :LC, :256].transpose(0, 2, 1)
    for k in range(2):
        sl = A2T[:, k * 128:(k + 1) * 128]
        flat[:, :, k * 1024:(k + 1) * 1024] = sl[:, :, 0:1024]
        flat[:, :, 6144 + k * 1024:6144 + (k + 1) * 1024] = sl[:, :, 1024:2048]
        for g in range(4):
            flat[:, :, OFF_G[g] + k * 512:OFF_G[g] + (k + 1) * 512] = \
                B2T[:, k * 128:(k + 1) * 128, g * 512:(g + 1) * 512]
    ux3 = np.empty((B, 64, 4096), np.float32)
    ux3[:, :, 0:LC] = A[:, :LC, 256:].transpose(0, 2, 1)
    ux3[:, :, LC:] = Bm[:, :LC, 256:].transpose(0, 2, 1)
    bf = mybir.dt.np(BF16)
    return A, Bm, flat.astype(bf), ux3.astype(bf)


def kernel(x, W, b, _trace=False):
    global last_results
    x = np.ascontiguousarray(np.asarray(x, dtype=np.float32))
    W = np.asarray(W, dtype=np.float32)
    b = np.asarray(b, dtype=np.float32)
    bias_val = float(b[0])

    if bias_val not in _cache:
        _cache.clear()
        _cache[bias_val] = _build(bias_val)
    nc = _cache[bias_val]

    A, Bm, flat, ux3 = _host_pack(x, W)
    in_maps = [{"inflat": flat[c], "ux3": ux3[c]} for c in range(N_CORES)]
    res = run_bass_kernel_spmd(nc, in_maps, core_ids=list(range(N_CORES)),
                               trace=_trace)
    last_results = res

    out = np.empty((B, L, L), dtype=np.float32)
    for c in range(N_CORES):
        out[c, :LC, :LC] = res.results[c]["out"].astype(np.float32)
    # host-computed tails: rows 2048:2050 (all j) and cols 2048:2050
    row_logits = np.einsum('bid,bjd->bij', A[:, LC:], Bm, optimize=True)
    col_logits = np.einsum('bid,bjd->bij', A[:, :LC], Bm[:, LC:],
                           optimize=True)
    out[:, LC:, :] = _sigmoid(row_logits + bias_val)
    out[:, :LC, LC:] = _sigmoid(col_logits + bias_val)
    return out


# revision 19
# speedup vs baseline: 1.1229x; 1.0780x over previous
"""ESM2 contact predictor head on 8 Trainium2 NeuronCores.

Computes out[b, i, j] = sigmoid(x[b,i] @ W @ x[b,j] + bias) for
x: (8, 2050, 320) f32, W: (320, 320) f32, bias: (1,) f32.

Sharding: data-parallel over batch — core c handles batch element c.

Decomposition (host + device):
  host:  SVD-rotate the bilinear form: W = U S V^T, A = x @ (U sqrt(S)),
         B = x @ (V sqrt(S)), so logits = A @ B^T. Host also computes the
         2 tail rows (i=2048:2050) and 2 tail cols (j=2048:2050) exactly,
         so the device handles a clean 2048x2048 grid.
  chip:  per core, out[0:2048, 0:2048] = sigmoid(A @ B^T + b) in bf16
         (K=320 as 2 x 128-mode slabs + 1 x 64-mode slab, all bf16 at
         1 col/cycle; fp8 DoubleRow was tried and pins the PE clock at
         the 1.2 GHz mid pstate — net loss).
         All input lands via 3 large DMAs into one flat SBUF buffer
         (per-DMA overhead ~1us makes small chunks wire-inefficient).
         13 warmup matmuls bridge the input-DMA window so the PE stream
         never gaps (a gap resets the 2.4 GHz pstate ramp). Strips 0-3
         are scheduled in input-arrival order (j-groups 0-1 first, the
         64-mode slab and j-groups 2-3 after the second DMA). PSUM is
         managed as 8 single-bank [128,512] regions; matmuls touching
         the same region are >=4 apart to hide the accumulate RMW
         turnaround. Fused sigmoid+bias on ScalarE writes bf16 strips;
         one 0.5 MB DMA out per 128-row strip (split for the last strip
         so the final drain is short).
"""

import numpy as np

import concourse.mybir as mybir
import concourse.tile as tile
from concourse import bacc
from concourse.bass_utils import run_bass_kernel_spmd

N_CORES = 8
B, L, D = 8, 2050, 320
LC = 2048          # device output grid (tails handled on host)
F32 = mybir.dt.float32
BF16 = mybir.dt.bfloat16
SIG = mybir.ActivationFunctionType.Sigmoid

# flat input layout, [128, FLAT] bf16 elements per partition:
#   [0:2048)      u2A: stationary A^T modes 0:256 as (k, i), i in 0:1024
#   [2048:3072)   x2 g0: moving B^T modes 0:256 as (k, j), j in 0:512
#   [3072:4096)   x2 g1
#   [4096:5120)   x2 g2
#   [5120:6144)   x2 g3
#   [6144:8192)   u2B: stationary, i in 1024:2048
# plus a [64, 4096] tensor for the 64-mode slab (lhsT/rhs must share a
# base partition): [0:2048) = A^T modes 256:320, [2048:4096) = B^T.
FLAT = 8192
OFF_U2 = (0, 6144)
OFF_G = (2048, 3072, 4096, 5120)

_cache = {}


def _build(bias_val: float):
    nc = bacc.Bacc("TRN2", target_bir_lowering=False, debug=False,
                   num_devices=N_CORES)
    in_d = nc.dram_tensor("inflat", [128, FLAT], BF16, kind="ExternalInput")
    ux3_d = nc.dram_tensor("ux3", [64, 4096], BF16, kind="ExternalInput")
    out_d = nc.dram_tensor("out", [LC, LC], BF16, kind="ExternalOutput")

    with tile.TileContext(nc) as tc:
        with (
            tc.tile_pool(name="persist", bufs=1) as pp,
            tc.tile_pool(name="outp", bufs=4) as outp,
            tc.tile_pool(name="psum", bufs=8, space="PSUM") as psp,
        ):
            bias_t = pp.tile([128, 1], F32)
            nc.vector.memset(bias_t[:], bias_val)

            inbuf = pp.tile([128, FLAT], BF16)
            ux3buf = pp.tile([64, 4096], BF16)

            def u2ap(k, i0):
                ih, off = divmod(i0, 1024)
                base = OFF_U2[ih] + k * 1024 + off
                return inbuf[:, base:base + 128]

            def x2ap(g, k):
                base = OFF_G[g] + k * 512
                return inbuf[:, base:base + 512]

            def u3ap(i0):
                return ux3buf[:, i0:i0 + 128]

            def x3ap(g):
                return ux3buf[:, 2048 + g * 512:2048 + (g + 1) * 512]

            # 4 large input DMAs on the SP queue (per-DMA overhead ~1us;
            # a second queue on the Activation engine evicts the sigmoid
            # table and delays ACTs — measured net loss)
            nc.sync.dma_start(inbuf[:, 0:4096], in_d.ap()[:, 0:4096])
            nc.sync.dma_start(ux3buf[:], ux3_d.ap())
            nc.sync.dma_start(inbuf[:, 4096:6144], in_d.ap()[:, 4096:6144])
            nc.sync.dma_start(inbuf[:, 6144:FLAT], in_d.ap()[:, 6144:FLAT])

            # PE warmup: release the HAM clock-gate and hold the pstate ramp
            # while DMA 1 (u2A + j-groups 0-1) lands.
            warm_sb = pp.tile([128, 512], BF16)
            nc.vector.memset(warm_sb.bitcast(F32)[:], 1.0)
            psw = [psp.tile([128, 512], F32, tag="reg", bufs=8,
                            name=f"psw{i}") for i in range(2)]
            for wi in range(13):
                nc.tensor.matmul(psw[wi % 2][:, :], lhsT=warm_sb[:, :128],
                                 rhs=warm_sb[:], start=True, stop=True)
            # preload the sigmoid activation table while DMAs run
            act_warm = pp.tile([128, 1], F32)
            nc.scalar.activation(act_warm[:], bias_t[:], SIG)

            strips = {}
            regs = {}

            def reg(s, r):
                if (s, r) not in regs:
                    regs[(s, r)] = psp.tile([128, 512], F32, tag="reg",
                                            bufs=8, name=f"r{s}_{r}")
                return regs[(s, r)]

            def mm2(s, r, k):
                nc.tensor.matmul(reg(s, r)[:, :], lhsT=u2ap(k, s * 128),
                                 rhs=x2ap(r, k), start=(k == 0), stop=False)

            def mm3(s, r):
                nc.tensor.matmul(reg(s, r)[:, :], lhsT=u3ap(s * 128),
                                 rhs=x3ap(r), start=False, stop=True)

            def act(s, r):
                nc.scalar.activation(strips[s][:, r * 512:(r + 1) * 512],
                                     reg(s, r)[:, :], SIG, bias=bias_t[:, :])
                del regs[(s, r)]

            def dma_out(s, r0, r1, eng=None):
                i0 = s * 128
                (eng or nc.sync).dma_start(
                    out_d.ap()[i0:i0 + 128, r0 * 512:r1 * 512],
                    strips[s][:, r0 * 512:r1 * 512])

            for s in range(4):
                strips[s] = outp.tile([128, LC], BF16, tag="strip", bufs=4,
                                      name=f"strip{s}")
            # lead-in: strips 0-3 in input-arrival order.
            # B1: j-groups 0-1 (needs DMA 1 only), distance-8
            for k in range(2):
                for s in range(4):
                    for r in range(2):
                        mm2(s, r, k)
            # B2: 64-mode slab for those regions (needs DMA 2), then sigmoid
            for s in range(4):
                for r in range(2):
                    mm3(s, r)
                    act(s, r)
            # B3: j-groups 2-3 (needs DMA 2), distance-8, then close strips
            for k in range(2):
                for s in range(4):
                    for r in range(2, 4):
                        mm2(s, r, k)
            for s in range(4):
                for r in range(2, 4):
                    mm3(s, r)
                    act(s, r)
                dma_out(s, 0, 4)

            # steady state: strips 4-15, distance-4 within a strip
            for s in range(4, 16):
                strips[s] = outp.tile([128, LC], BF16, tag="strip", bufs=4,
                                      name=f"strip{s}")
                for k in range(2):
                    for r in range(4):
                        mm2(s, r, k)
                for r in range(4):
                    mm3(s, r)
                    act(s, r)
                    if s == 15 and r == 1:
                        dma_out(s, 0, 2)   # split the last strip's DMA
                if s == 15:
                    dma_out(s, 2, 4, eng=nc.sync)
                else:
                    dma_out(s, 0, 4)

    nc.compile()
    return nc


last_results = None


def _sigmoid(z):
    return 1.0 / (1.0 + np.exp(-z))


def _host_pack(x, W):
    U, s, Vt = np.linalg.svd(W.astype(np.float64))
    A = np.ascontiguousarray(x @ (U * np.sqrt(s)).astype(np.float32))
    Bm = np.ascontiguousarray(x @ (Vt.T * np.sqrt(s)).astype(np.float32))

    flat = np.empty((B, 128, FLAT), np.float32)
    A2T = A[:, :LC, :256].transpose(0, 2, 1)       # (B, 256, 2048)
    B2T = Bm[:, :LC, :256].transpose(0, 2, 1)
    for k in range(2):
        sl = A2T[:, k * 128:(k + 1) * 128]
        flat[:, :, k * 1024:(k + 1) * 1024] = sl[:, :, 0:1024]
        flat[:, :, 6144 + k * 1024:6144 + (k + 1) * 1024] = sl[:, :, 1024:2048]
        for g in range(4):
            flat[:, :, OFF_G[g] + k * 512:OFF_G[g] + (k + 1) * 512] = \
                B2T[:, k * 128:(k + 1) * 128, g * 512:(g + 1) * 512]
    ux3 = np.empty((B, 64, 4096), np.float32)
    ux3[:, :, 0:LC] = A[:, :LC, 256:].transpose(0, 2, 1)
    ux3[:, :, LC:] = Bm[:, :LC, 256:].transpose(0, 2, 1)
    bf = mybir.dt.np(BF16)
    return A, Bm, flat.astype(bf), ux3.astype(bf)


def kernel(x, W, b, _trace=False):
    global last_results
    x = np.ascontiguousarray(np.asarray(x, dtype=np.float32))
    W = np.asarray(W, dtype=np.float32)
    b = np.asarray(b, dtype=np.float32)
    bias_val = float(b[0])

    if bias_val not in _cache:
        _cache.clear()
        _cache[bias_val] = _build(bias_val)
    nc = _cache[bias_val]

    A, Bm, flat, ux3 = _host_pack(x, W)
    in_maps = [{"inflat": flat[c], "ux3": ux3[c]} for c in range(N_CORES)]
    res = run_bass_kernel_spmd(nc, in_maps, core_ids=list(range(N_CORES)),
                               trace=_trace)
    last_results = res

    out = np.empty((B, L, L), dtype=np.float32)
    for c in range(N_CORES):
        out[c, :LC, :LC] = res.results[c]["out"].astype(np.float32)
    # host-computed tails: rows 2048:2050 (all j) and cols 2048:2050
    row_logits = np.einsum('bid,bjd->bij', A[:, LC:], Bm, optimize=True)
    col_logits = np.einsum('bid,bjd->bij', A[:, :LC], Bm[:, LC:],
                           optimize=True)
    out[:, LC:, :] = _sigmoid(row_logits + bias_val)
    out[:, :LC, LC:] = _sigmoid(col_logits + bias_val)
    return out
